# revision 1
# baseline (speedup 1.0000x reference)
"""Trainium2 Bass kernel for DSSConv2d (low-rank spatially-combined 3x3 conv).

Computation (per reference):
  convs = conv2d(x, w.reshape(rank*oc, ic, 3, 3), pad=1)   # [B, rank*oc, H, W]
  cw    = softmax(cw_row + cw_col, axis=0)                 # [rank, H, W]
  out   = einsum('bkcxy,kxy->bcxy', convs.reshape(B,rank,oc,H,W), cw)
  out  += b + b_col + b_row

Strategy:
  - Data parallel: batch 32 -> 4 images per core on 8 cores.
  - 3x3 conv = 9 shifted 1x1 convs: for each (dy,dx), a [ic,oc] matmul over a
    shifted slice of zero-padded x, accumulated in PSUM. bf16 inputs, fp32 acc.
  - Layout: psum[oc, pix] with weights stationary; pixel tiles = 8 image rows
    (N = 8*56 = 448 <= 512 PSUM bank).
  - Per-pixel rank combine on VectorE with the softmax weights broadcast
    across partitions via a stride-0 DMA; bias plane precomputed on host.
"""

import numpy as np
import ml_dtypes
from contextlib import ExitStack

import concourse.bass as bass
import concourse.mybir as mybir
import concourse.tile as tile
from concourse import bacc
from concourse.bass_utils import run_bass_kernel_spmd

RANK, OC, IC = 4, 128, 128
B, H, W = 32, 56, 56
NCORES = 8
B_LOC = B // NCORES          # 4 images per core
HP, WP = H + 2, W + 2        # zero-padded input: 58 x 58
PIX = H * W                  # 3136
RB = 8                       # output rows per pixel tile
NBLK = H // RB               # 7 tiles per image
NT = RB * W                  # 448 pixels per tile

BF16 = mybir.dt.bfloat16
F32 = mybir.dt.float32

_CACHE = {}


def _build_nc():
    nc = bacc.Bacc()
    x_in = nc.dram_tensor("x", [B_LOC, IC, HP, WP], BF16, kind="ExternalInput")
    w_in = nc.dram_tensor("w", [IC, RANK, 9, OC], BF16, kind="ExternalInput")
    cw_in = nc.dram_tensor("cw", [RANK, PIX], BF16, kind="ExternalInput")
    bias_in = nc.dram_tensor("bias", [OC, PIX], F32, kind="ExternalInput")
    out = nc.dram_tensor("out", [B_LOC, OC, PIX], F32, kind="ExternalOutput")

    with tile.TileContext(nc) as tc, ExitStack() as ctx:
        consts = ctx.enter_context(tc.tile_pool(name="consts", bufs=1))
        xpool = ctx.enter_context(tc.tile_pool(name="xpool", bufs=3))
        mpool = ctx.enter_context(tc.tile_pool(name="mpool", bufs=3))
        opool = ctx.enter_context(tc.tile_pool(name="opool", bufs=4))
        pspool = ctx.enter_context(tc.tile_pool(name="ps", bufs=8, space="PSUM"))

        # Row blocks per image: 8 rows -> matmul N = 448 (<=512 ISA limit).
        BLKS = [(h0, 8) for h0 in range(0, H, 8)]
        NTMAX = 8 * W

        # x in standalone row chunks per image (rows incl. 3x3 halo) so the
        # first matmuls wait on a ~0.26 MB transfer, not a full image.
        XCH = [(0, 18), (16, 18), (32, 18), (48, 10)]

        def load_img(img, eng, layout):
            chunks = []
            for ci, (r0, nr) in enumerate(layout):
                xc = xpool.tile([IC, nr, WP], BF16, name=f"x{ci}", tag=f"x{ci}")
                eng.dma_start(out=xc, in_=x_in[img][:, r0 : r0 + nr, :])
                chunks.append(xc)
            return chunks

        def chunk_for(layout, h0, nr):
            # deepest chunk that covers padded rows [h0, h0+nr+2)
            for ci in range(len(layout) - 1, -1, -1):
                r0, cnr = layout[ci]
                if r0 <= h0 and h0 + nr + 2 <= r0 + cnr:
                    return ci
            raise AssertionError((h0, nr))

        # Startup critical path: the first matmul needs only w[r0] and the
        # first x chunk. Issue those first, on separate HWDGE rings
        # (sync / scalar) so descriptor generation runs in parallel.
        w_sb = []
        w0 = consts.tile([IC, 9, OC], BF16, name="w0")
        nc.sync.dma_start(out=w0, in_=w_in[:, 0, :, :])
        w_sb.append(w0)

        x0 = load_img(0, nc.scalar, XCH)

        for r in range(1, RANK):
            wr = consts.tile([IC, 9, OC], BF16, name=f"w{r}")
            nc.sync.dma_start(out=wr, in_=w_in[:, r, :, :])
            w_sb.append(wr)

        # softmax combine weights (bf16), replicated across all 128
        # partitions; per-rank tiles so the first combine only waits on
        # rank 0's 0.8 MB broadcast, not the whole 3.2 MB
        cwb = []
        for r in range(RANK):
            cr = consts.tile([128, PIX], BF16, name=f"cw{r}")
            nc.sync.dma_start(out=cr, in_=cw_in[r].partition_broadcast(128))
            cwb.append(cr)

        bias_sb = consts.tile([OC, PIX], F32)
        nc.scalar.dma_start(out=bias_sb, in_=bias_in[:, :])

        # HAM warmup: ~3.4 us of dummy matmuls fill the preamble->first-DMA
        # idle window so real matmuls start at 2.4 GHz. A throwaway reader
        # releases the PSUM slot back to the pool before the main loop
        # needs all 8 banks.
        warm = consts.tile([128, NTMAX], BF16, name="warm")
        nc.gpsimd.memset(warm, 0.0)
        wps = pspool.tile([OC, NTMAX], F32, name="wps", tag="ps")
        for _ in range(10):
            nc.tensor.matmul(
                wps[:, :], lhsT=warm[:, :128], rhs=warm, start=True, stop=True
            )
        wsink = consts.tile([128, 1], F32, name="wsink")
        nc.vector.tensor_copy(wsink, wps[:, 0:1])

        x_cur, lay_cur = x0, XCH
        for img in range(B_LOC):
            if img + 1 < B_LOC:
                x_nxt = load_img(img + 1, nc.scalar, XCH)
            blks = BLKS
            if img == B_LOC - 1:
                # shorter final blocks -> shorter post-matmul DVE/DMA tail
                blks = BLKS[:-1] + [(48, 4), (52, 4)]
            for blk, (h0, nr) in enumerate(blks):
                ci = chunk_for(lay_cur, h0, nr)
                xc = x_cur[ci]
                hoff = h0 - lay_cur[ci][0]
                nt = nr * W
                p0 = h0 * W
                psums = []
                for r in range(RANK):
                    ps = pspool.tile(
                        [OC, NTMAX], F32, name=f"ps{r}", tag="ps"
                    )
                    for o in range(9):
                        dy, dx = o // 3, o % 3
                        nc.tensor.matmul(
                            ps[:, :nt],
                            lhsT=w_sb[r][:, o, :],
                            rhs=xc[:, hoff + dy : hoff + dy + nr, dx : dx + W],
                            start=(o == 0),
                            stop=(o == 8),
                        )
                    psums.append(ps)
                # combine: eager chain with the bias folded into the first
                # add, so the DVE tail after the last matmul is only
                # (mul3, final add)
                acc = None
                for r in range(RANK):
                    m = mpool.tile([OC, NTMAX], F32, name=f"m{r}", tag=f"m{r}")
                    nc.vector.tensor_mul(
                        m[:, :nt], psums[r][:, :nt], cwb[r][:, p0 : p0 + nt]
                    )
                    if acc is None:
                        b0 = mpool.tile([OC, NTMAX], F32, name="b0", tag="b0")
                        nc.vector.tensor_add(
                            b0[:, :nt], m[:, :nt], bias_sb[:, p0 : p0 + nt]
                        )
                        acc = b0
                    else:
                        dst = (
                            opool.tile([OC, NTMAX], F32, name="ot", tag="ot")
                            if r == RANK - 1
                            else mpool.tile(
                                [OC, NTMAX], F32, name=f"a{r}", tag=f"a{r}"
                            )
                        )
                        nc.vector.tensor_add(
                            dst[:, :nt], acc[:, :nt], m[:, :nt]
                        )
                        acc = dst
                nc.sync.dma_start(
                    out=out[img][:, p0 : p0 + nt], in_=acc[:, :nt]
                )
            if img + 1 < B_LOC:
                x_cur, lay_cur = x_nxt, XCH
    nc.finalize()
    return nc


def _prep_inputs(x, w, cw_row, cw_col, b_row, b_col, b):
    # zero-padded bf16 input
    xp = np.zeros((B, IC, HP, WP), dtype=ml_dtypes.bfloat16)
    xp[:, :, 1 : H + 1, 1 : W + 1] = x.astype(ml_dtypes.bfloat16)

    # weights: [rank, oc, ic, kh, kw] -> [ic, rank, kh*kw, oc], bf16
    wt = np.ascontiguousarray(np.transpose(w, (2, 0, 3, 4, 1))).reshape(
        IC, RANK, 9, OC
    ).astype(ml_dtypes.bfloat16)

    # softmax over rank of per-pixel combine logits
    logits = (cw_row + cw_col).astype(np.float64)  # [rank, H, W]
    logits -= logits.max(axis=0, keepdims=True)
    e = np.exp(logits)
    cw = (
        (e / e.sum(axis=0, keepdims=True))
        .astype(ml_dtypes.bfloat16)
        .reshape(RANK, PIX)
    )

    # combined bias plane [oc, pix]
    bias = (
        b.reshape(OC, 1, 1) + b_row.reshape(1, H, 1) + b_col.reshape(1, 1, W)
    ).astype(np.float32).reshape(OC, PIX)

    return xp, wt, cw, bias


def _run(inputs, trace=False):
    if "nc" not in _CACHE:
        _CACHE["nc"] = _build_nc()
    nc = _CACHE["nc"]
    xp, wt, cw, bias = _prep_inputs(**inputs)
    in_maps = [
        {"x": xp[c * B_LOC : (c + 1) * B_LOC], "w": wt, "cw": cw, "bias": bias}
        for c in range(NCORES)
    ]
    res = run_bass_kernel_spmd(nc, in_maps, list(range(NCORES)), trace=trace)
    outs = [np.asarray(res.results[c]["out"]) for c in range(NCORES)]
    full = np.concatenate(outs, axis=0).reshape(B, OC, H, W).astype(np.float32)
    return full, res


def kernel(**inputs):
    full, _ = _run(inputs)
    return full



# revision 2
# speedup vs baseline: 1.2007x; 1.2007x over previous
"""Trainium2 Bass kernel for DSSConv2d (low-rank spatially-combined 3x3 conv).

Computation (per reference):
  convs = conv2d(x, w.reshape(rank*oc, ic, 3, 3), pad=1)   # [B, rank*oc, H, W]
  cw    = softmax(cw_row + cw_col, axis=0)                 # [rank, H, W]
  out   = einsum('bkcxy,kxy->bcxy', convs.reshape(B,rank,oc,H,W), cw)
  out  += b + b_col + b_row

Strategy:
  - Data parallel: batch 32 -> 4 images per core on 8 cores.
  - 3x3 conv = 9 shifted 1x1 convs: for each (dy,dx), a [ic,oc] matmul over a
    shifted slice of zero-padded x, accumulated in PSUM. bf16 inputs, fp32 acc.
  - Layout: psum[oc, pix] with weights stationary; pixel tiles = 8 image rows
    (N = 8*56 = 448 <= 512 PSUM bank).
  - Per-pixel rank combine on VectorE with the softmax weights broadcast
    across partitions via a stride-0 DMA; bias plane precomputed on host.
"""

import numpy as np
import ml_dtypes
from contextlib import ExitStack

import concourse.bass as bass
import concourse.mybir as mybir
import concourse.tile as tile
from concourse import bacc
from concourse.bass_utils import run_bass_kernel_spmd

RANK, OC, IC = 4, 128, 128
B, H, W = 32, 56, 56
NCORES = 8
B_LOC = B // NCORES          # 4 images per core
HP, WP = H + 2, W + 2        # zero-padded input: 58 x 58
PIX = H * W                  # 3136
RB = 8                       # output rows per pixel tile
NBLK = H // RB               # 7 tiles per image
NT = RB * W                  # 448 pixels per tile

BF16 = mybir.dt.bfloat16
F32 = mybir.dt.float32

_CACHE = {}


def _build_nc():
    nc = bacc.Bacc()
    x_in = nc.dram_tensor("x", [B_LOC, IC, HP, WP], BF16, kind="ExternalInput")
    w_in = nc.dram_tensor("w", [IC, RANK, 9, OC], BF16, kind="ExternalInput")
    cw_in = nc.dram_tensor("cw", [RANK, PIX], BF16, kind="ExternalInput")
    bias_in = nc.dram_tensor("bias", [OC, PIX], BF16, kind="ExternalInput")
    out = nc.dram_tensor("out", [B_LOC, OC, PIX], BF16, kind="ExternalOutput")

    with tile.TileContext(nc) as tc, ExitStack() as ctx:
        consts = ctx.enter_context(tc.tile_pool(name="consts", bufs=1))
        xpool = ctx.enter_context(tc.tile_pool(name="xpool", bufs=3))
        mpool = ctx.enter_context(tc.tile_pool(name="mpool", bufs=3))
        opool = ctx.enter_context(tc.tile_pool(name="opool", bufs=4))
        pspool = ctx.enter_context(tc.tile_pool(name="ps", bufs=8, space="PSUM"))

        # Row blocks per image: 8 rows -> matmul N = 448 (<=512 ISA limit).
        BLKS = [(h0, 8) for h0 in range(0, H, 8)]
        NTMAX = 8 * W

        # x in standalone row chunks per image (rows incl. 3x3 halo) so the
        # first matmuls wait on a ~0.26 MB transfer, not a full image.
        XCH = [(0, 18), (16, 18), (32, 18), (48, 10)]
        # image 0 split finer and spread across rings: first block only
        # needs padded rows [0, 10)
        XCH0 = [(0, 10), (8, 10), (16, 18), (32, 18), (48, 10)]

        RINGS = [nc.sync, nc.scalar, nc.vector, nc.gpsimd]

        def load_img(img, layout, spread=False):
            chunks = []
            for ci, (r0, nr) in enumerate(layout):
                xc = xpool.tile([IC, nr, WP], BF16, name=f"x{ci}", tag=f"x{ci}")
                eng = RINGS[ci % len(RINGS)] if spread else nc.scalar
                eng.dma_start(out=xc, in_=x_in[img][:, r0 : r0 + nr, :])
                chunks.append(xc)
            return chunks

        def chunk_for(layout, h0, nr):
            # deepest chunk that covers padded rows [h0, h0+nr+2)
            for ci in range(len(layout) - 1, -1, -1):
                r0, cnr = layout[ci]
                if r0 <= h0 and h0 + nr + 2 <= r0 + cnr:
                    return ci
            raise AssertionError((h0, nr))

        # Startup critical path: the first matmuls need only w[rank0] and
        # the first x chunk. Rank 0's weights go per-tap across all four
        # HWDGE rings so tap o's matmul waits on a 32 KB transfer.
        w_sb = []
        w0 = consts.tile([IC, 9, OC], BF16, name="w0")
        for o in range(9):
            RINGS[o % 4].dma_start(out=w0[:, o, :], in_=w_in[:, 0, o, :])
        w_sb.append(w0)

        x0 = load_img(0, XCH0, spread=True)

        for r in range(1, RANK):
            wr = consts.tile([IC, 9, OC], BF16, name=f"w{r}")
            RINGS[r % 4].dma_start(out=wr, in_=w_in[:, r, :, :])
            w_sb.append(wr)

        # softmax combine weights (bf16), replicated across all 128
        # partitions; per-rank tiles so the first combine only waits on
        # rank 0's 0.8 MB broadcast, not the whole 3.2 MB
        cwb = []
        for r in range(RANK):
            cr = consts.tile([128, PIX], BF16, name=f"cw{r}")
            RINGS[r % 4].dma_start(out=cr, in_=cw_in[r].partition_broadcast(128))
            cwb.append(cr)

        bias_sb = consts.tile([OC, PIX], BF16)
        nc.scalar.dma_start(out=bias_sb, in_=bias_in[:, :])

        # HAM warmup: dummy matmuls from t~0.3us cover the HAM cold window
        # and the DMA preamble, so real matmuls start at full clock. The
        # warm tile comes from a DVE memset (fast; GPSIMD ucode boot used
        # to delay this by ~6us). A throwaway reader releases the PSUM
        # slot back to the pool before the main loop needs all 8 banks.
        warm = consts.tile([128, NTMAX], BF16, name="warm")
        nc.vector.memset(warm, 0.0)
        wps = pspool.tile([OC, NTMAX], F32, name="wps", tag="ps")
        for _ in range(10):
            nc.tensor.matmul(
                wps[:, :], lhsT=warm[:, :128], rhs=warm, start=True, stop=True
            )
        wsink = consts.tile([128, 1], F32, name="wsink")
        nc.vector.tensor_copy(wsink, wps[:, 0:1])

        x_cur, lay_cur = x0, XCH0
        for img in range(B_LOC):
            if img + 1 < B_LOC:
                x_nxt = load_img(img + 1, XCH)
            blks = BLKS
            if img == B_LOC - 1:
                # shorter final blocks -> shorter post-matmul DVE/DMA tail
                blks = BLKS[:-1] + [(48, 4), (52, 2), (54, 2)]
            for blk, (h0, nr) in enumerate(blks):
                ci = chunk_for(lay_cur, h0, nr)
                xc = x_cur[ci]
                hoff = h0 - lay_cur[ci][0]
                nt = nr * W
                p0 = h0 * W
                psums = []
                for r in range(RANK):
                    ps = pspool.tile(
                        [OC, NTMAX], F32, name=f"ps{r}", tag="ps"
                    )
                    for o in range(9):
                        dy, dx = o // 3, o % 3
                        nc.tensor.matmul(
                            ps[:, :nt],
                            lhsT=w_sb[r][:, o, :],
                            rhs=xc[:, hoff + dy : hoff + dy + nr, dx : dx + W],
                            start=(o == 0),
                            stop=(o == 8),
                        )
                    psums.append(ps)
                # combine: eager chain with the bias folded into the first
                # add. Products and partial sums are bf16 so the adds run
                # in the DVE 2x packed mode and the output DMA halves.
                acc = None
                for r in range(RANK):
                    m = mpool.tile([OC, NTMAX], BF16, name=f"m{r}", tag=f"m{r}")
                    nc.vector.tensor_mul(
                        m[:, :nt], psums[r][:, :nt], cwb[r][:, p0 : p0 + nt]
                    )
                    if acc is None:
                        b0 = mpool.tile([OC, NTMAX], BF16, name="b0", tag="b0")
                        nc.vector.tensor_add(
                            b0[:, :nt], m[:, :nt], bias_sb[:, p0 : p0 + nt]
                        )
                        acc = b0
                    else:
                        dst = (
                            opool.tile([OC, NTMAX], BF16, name="ot", tag="ot")
                            if r == RANK - 1
                            else mpool.tile(
                                [OC, NTMAX], BF16, name=f"a{r}", tag=f"a{r}"
                            )
                        )
                        nc.vector.tensor_add(
                            dst[:, :nt], acc[:, :nt], m[:, :nt]
                        )
                        acc = dst
                RINGS[blk % 2].dma_start(
                    out=out[img][:, p0 : p0 + nt], in_=acc[:, :nt]
                )
            if img + 1 < B_LOC:
                x_cur, lay_cur = x_nxt, XCH
    nc.finalize()
    return nc


def _prep_inputs(x, w, cw_row, cw_col, b_row, b_col, b):
    # zero-padded bf16 input
    xp = np.zeros((B, IC, HP, WP), dtype=ml_dtypes.bfloat16)
    xp[:, :, 1 : H + 1, 1 : W + 1] = x.astype(ml_dtypes.bfloat16)

    # weights: [rank, oc, ic, kh, kw] -> [ic, rank, kh*kw, oc], bf16
    wt = np.ascontiguousarray(np.transpose(w, (2, 0, 3, 4, 1))).reshape(
        IC, RANK, 9, OC
    ).astype(ml_dtypes.bfloat16)

    # softmax over rank of per-pixel combine logits
    logits = (cw_row + cw_col).astype(np.float64)  # [rank, H, W]
    logits -= logits.max(axis=0, keepdims=True)
    e = np.exp(logits)
    cw = (
        (e / e.sum(axis=0, keepdims=True))
        .astype(ml_dtypes.bfloat16)
        .reshape(RANK, PIX)
    )

    # combined bias plane [oc, pix]
    bias = (
        b.reshape(OC, 1, 1) + b_row.reshape(1, H, 1) + b_col.reshape(1, 1, W)
    ).astype(np.float32).reshape(OC, PIX)

    return xp, wt, cw, bias


def _run(inputs, trace=False):
    if "nc" not in _CACHE:
        _CACHE["nc"] = _build_nc()
    nc = _CACHE["nc"]
    xp, wt, cw, bias = _prep_inputs(**inputs)
    in_maps = [
        {"x": xp[c * B_LOC : (c + 1) * B_LOC], "w": wt, "cw": cw, "bias": bias}
        for c in range(NCORES)
    ]
    res = run_bass_kernel_spmd(nc, in_maps, list(range(NCORES)), trace=trace)
    outs = [np.asarray(res.results[c]["out"]) for c in range(NCORES)]
    full = np.concatenate(outs, axis=0).reshape(B, OC, H, W).astype(np.float32)
    return full, res


def kernel(**inputs):
    full, _ = _run(inputs)
    return full

